# revision 19
# baseline (speedup 1.0000x reference)
"""Trainium2 Bass kernel for CentroidPool (retrieval_knn).

Problem: latent [65536, 128] f32, coords [4096, 128] f32.
Output: closest_centroid [65536] int32 = argmin_k ||latent_n - coords_k||.

Math: argmin_k ||x-c_k|| = argmax_k (x . c_k - 0.5*||c_k||^2).  Data-parallel
over N across 8 cores, coords replicated.

PE (2 fp16 passes per 512-col PSUM bank):
  score = xh.ch + xl'.cb  where xh = fp16(x), xl' = fp16(x - xh) with contract
  rows 126/127 replaced by ones, and cb = fp16(c) with rows 126/127 replaced
  by the hi/lo split of -0.5*||c||^2.  (The dropped xh.cl cross term and the
  two dropped xl rows shift scores by ~1e-3; measured against the fp64
  reference this flips 85/65536 argmaxes = 8.5e-4 harness rel err, 23x under
  the 2e-2 gate.)

Argmax (the old DVE bottleneck, max8+find_index8 = 2 full fp32 scans):
  - ACT copies PSUM->SBUF as fp16 (sc16), so DVE tensor ops run in 2x mode
  - DVE: 4-level tensor_max tree over 16-elem groups -> gmax[128,256]
    (2048+512+256+256 cycles instead of a 4096-cycle 1x scan)
  - DVE: max8 + max_index on gmax (256) -> max m + first group g* holding it
  - GPSIMD ap_gather: gather group g* (16 elems) for every row (indices are
    shared per 16-partition cluster, so each row receives all 16 cluster
    rows' groups = 256 gathered values, all slices of its own score row; its
    own argmax group is always among them)
  - DVE: max_index over the gathered 256 -> position of m; host decodes k.
All value comparisons happen on the same fp16-rounded scores (monotone), so
the pipeline is self-consistent; ties resolve to the first occurrence except
across exact fp16 ties (counted in the 85 flips above).

Built with bacc.Bacc + compile() so multi-wait instructions are legalized.
NOTE: tensor_tensor_reduce is avoided - it wedges TRN2 (NRT unrecoverable).
NOTE: fp32r matmul is avoided - TF32-class precision flips argmins.
NOTE: InstPool is avoided - walrus codegen rejects its 5D AP encoding
      (is_valid_s4d4_pl_addr) on this toolchain.
NOTE: gpsimd indirect_copy is avoided - it dies on HW (INTERNAL error on
      result fetch); ap_gather is the working path.
NOTE: gpsimd tensor_tensor is avoided - TENSOR_TENSOR is not a legal Pool
      engine opcode (walrus NCC_IXCG966); GPSIMD also cannot access PSUM.

Measured on trn2 (8 cores): 297,476 ns HW exec (baseline max8+find_index8
fp32 design: 602,244 ns -> 2.02x).  130/65536 index mismatches = 1.88e-3
harness rel err (gate 2e-2).  Engine busy per 4.65us tile slot: DVE 4.05us
(pacer, zero steady-state gaps), ACT 3.87us (2 PSUM->SBUF fp16 copies -
the structural floor), PE 3.8us (16 matmuls), GPSIMD ~1us (gather).
Pipeline head ~24us, tail ~9us.
"""

import numpy as np

N, K, D = 65536, 4096, 128
NCORES = 8
NSHARD = N // NCORES          # 8192 rows per core
NTILES = NSHARD // 128        # 64 tiles of 128 rows
KHALF = K // 2                # 2048 = 4 PSUM banks
GSIZE = 16                    # argmax group size
NGRP = K // GSIZE             # 256 groups


def build_program(ntiles=NTILES):
    import concourse.mybir as mybir
    import concourse.tile as tile
    from concourse import bacc

    f32 = mybir.dt.float32
    f16 = mybir.dt.float16
    i16 = mybir.dt.int16
    u16 = mybir.dt.uint16

    nshard = ntiles * 128
    nc = bacc.Bacc("TRN2", target_bir_lowering=False, debug=False)
    xh_d = nc.dram_tensor("xh", [D, nshard], f16, kind="ExternalInput").ap()
    xo_d = nc.dram_tensor("xo", [D, nshard], f16, kind="ExternalInput").ap()
    ch_d = nc.dram_tensor("ch", [D, K], f16, kind="ExternalInput").ap()
    cb_d = nc.dram_tensor("cb", [D, K], f16, kind="ExternalInput").ap()
    outg = nc.dram_tensor("gidx", [128, ntiles * 8], u16, kind="ExternalOutput").ap()
    outp = nc.dram_tensor("pos", [128, ntiles * 8], u16, kind="ExternalOutput").ap()

    def grp(ap, j):
        return ap.rearrange("p (g j) -> p g j", j=j)

    with tile.TileContext(nc) as tc:
        with (
            tc.tile_pool(name="const", bufs=1) as constp,
            tc.tile_pool(name="xin", bufs=4) as xinp,
            tc.tile_pool(name="psum", bufs=2, space="PSUM") as psump,
            tc.tile_pool(name="scores", bufs=3) as scp,
            tc.tile_pool(name="tree", bufs=3) as treep,
            tc.tile_pool(name="gmax", bufs=3) as gmaxp,
            tc.tile_pool(name="m8", bufs=3) as m8p,
            tc.tile_pool(name="gat", bufs=3) as gatp,
            tc.tile_pool(name="iall", bufs=1) as iallp,
        ):
            # tile 0's inputs first in the DMA queue, then the first column
            # chunk of each constant, then the rest: the first matmuls need
            # only xh0/xo0 + chunk 0, so PE starts ~1.5us in instead of
            # waiting for the full 2MB constant load.
            xh0 = xinp.tile([D, 128], f16, tag="xh")
            nc.sync.dma_start(xh0[:], xh_d[:, 0:128])
            xo0 = xinp.tile([D, 128], f16, tag="xo")
            nc.sync.dma_start(xo0[:], xo_d[:, 0:128])
            ch_sb = constp.tile([D, K], f16)
            cb_sb = constp.tile([D, K], f16)
            nc.sync.dma_start(ch_sb[:, 0:512], ch_d[:, 0:512])
            nc.sync.dma_start(cb_sb[:, 0:512], cb_d[:, 0:512])
            for b in range(1, 8):
                s = slice(b * 512, (b + 1) * 512)
                nc.sync.dma_start(ch_sb[:, s], ch_d[:, s])
                nc.sync.dma_start(cb_sb[:, s], cb_d[:, s])
            gall = iallp.tile([128, ntiles * 8], u16)
            pall = iallp.tile([128, ntiles * 8], u16)

            for i in range(ntiles):
                if i == 0:
                    xh, xo = xh0, xo0
                else:
                    xh = xinp.tile([D, 128], f16, tag="xh")
                    nc.sync.dma_start(xh[:], xh_d[:, i * 128:(i + 1) * 128])
                    xo = xinp.tile([D, 128], f16, tag="xo")
                    nc.sync.dma_start(xo[:], xo_d[:, i * 128:(i + 1) * 128])
                sc = scp.tile([128, K], f16)
                for half in range(2):
                    ps = psump.tile([128, KHALF], f32)
                    koff = half * KHALF
                    for b in range(4):
                        pb = ps[:, b * 512:(b + 1) * 512]
                        co = koff + b * 512
                        nc.tensor.matmul(pb, xh[:], ch_sb[:, co:co + 512],
                                         start=True, stop=False)
                        nc.tensor.matmul(pb, xo[:], cb_sb[:, co:co + 512],
                                         start=False, stop=True)
                    nc.scalar.copy(sc[:, koff:koff + KHALF], ps[:])
                t1 = treep.tile([128, K // 2], f16, tag="t1")
                nc.vector.tensor_max(
                    grp(t1[:], 8), grp(sc[:], 16)[:, :, 0:8], grp(sc[:], 16)[:, :, 8:16]
                )
                t2 = treep.tile([128, K // 4], f16, tag="t2")
                nc.vector.tensor_max(
                    grp(t2[:], 4), grp(t1[:], 8)[:, :, 0:4], grp(t1[:], 8)[:, :, 4:8]
                )
                t3 = treep.tile([128, K // 8], f16, tag="t3")
                nc.vector.tensor_max(
                    grp(t3[:], 2), grp(t2[:], 4)[:, :, 0:2], grp(t2[:], 4)[:, :, 2:4]
                )
                gmax = gmaxp.tile([128, NGRP], f16)
                nc.vector.tensor_max(
                    grp(gmax[:], 1), grp(t3[:], 2)[:, :, 0:1], grp(t3[:], 2)[:, :, 1:2]
                )
                m8 = m8p.tile([128, 8], f16)
                nc.vector.max(m8[:], gmax[:])
                nc.vector.max_index(gall[:, i * 8:(i + 1) * 8], m8[:], gmax[:])
                gat = gatp.tile([128, NGRP], f16)
                nc.gpsimd.ap_gather(
                    grp(gat[:], GSIZE), grp(sc[:], GSIZE),
                    gall[:, i * 8:i * 8 + 1].bitcast(i16),
                    channels=128, num_elems=NGRP, d=GSIZE, num_idxs=GSIZE,
                )
                nc.vector.max_index(pall[:, i * 8:(i + 1) * 8], m8[:], gat[:])
            nc.sync.dma_start(outg[:], gall[:])
            nc.sync.dma_start(outp[:], pall[:])
    nc.compile()
    return nc


def make_inputs(latent, coords):
    latent = np.asarray(latent, dtype=np.float32)
    coords = np.asarray(coords, dtype=np.float32)
    xT = np.ascontiguousarray(latent.T)                      # [128, N] f32
    cT = np.ascontiguousarray(coords.T)                      # [128, K] f32
    xh = xT.astype(np.float16)
    xl = (xT - xh.astype(np.float32)).astype(np.float16)
    ch = cT.astype(np.float16)
    c2 = (coords * coords).sum(axis=1, dtype=np.float32)     # [K]
    bias = (-0.5 * c2).astype(np.float32)
    bh = bias.astype(np.float16)
    bl = (bias - bh.astype(np.float32)).astype(np.float16)
    # xo: xl with contract rows 126/127 replaced by ones (bias carriers)
    xo = xl.copy()
    xo[126:128, :] = np.float16(1.0)
    # cb: ch with rows 126/127 replaced by bias hi/lo
    cb = ch.copy()
    cb[126, :] = bh
    cb[127, :] = bl
    in_maps = []
    for c in range(NCORES):
        s = slice(c * NSHARD, (c + 1) * NSHARD)
        in_maps.append({
            "xh": np.ascontiguousarray(xh[:, s]),
            "xo": np.ascontiguousarray(xo[:, s]),
            "ch": ch, "cb": cb,
        })
    return in_maps


def gather_output(results, ntiles=NTILES):
    outs = []
    for c in range(NCORES):
        g = np.asarray(results[c]["gidx"]).reshape(128, ntiles, 8)[:, :, 0]
        pos = np.asarray(results[c]["pos"]).reshape(128, ntiles, 8)[:, :, 0]
        i = (pos >> 4).astype(np.int64)                      # which row's group
        j = (pos & 15).astype(np.int64)                      # offset within group
        p = np.arange(128)[:, None]
        donor = (p // 16) * 16 + i                           # [128, ntiles]
        grp = g[donor, np.arange(ntiles)[None, :]].astype(np.int64)
        k = grp * GSIZE + j                                  # [128, ntiles]
        outs.append(k.T.reshape(-1))                         # shard-row order
    return np.concatenate(outs).astype(np.int32)


_NC_CACHE = None


def kernel(latent, coords):
    global _NC_CACHE
    from concourse import bass_utils

    if _NC_CACHE is None:
        _NC_CACHE = build_program()
    in_maps = make_inputs(latent, coords)
    res = bass_utils.run_bass_kernel_spmd(
        _NC_CACHE, in_maps, core_ids=list(range(NCORES))
    )
    return gather_output(res.results)


# revision 21
# speedup vs baseline: 1.0028x; 1.0028x over previous
"""Trainium2 Bass kernel for CentroidPool (retrieval_knn).

Problem: latent [65536, 128] f32, coords [4096, 128] f32.
Output: closest_centroid [65536] int32 = argmin_k ||latent_n - coords_k||.

Math: argmin_k ||x-c_k|| = argmax_k (x . c_k - 0.5*||c_k||^2).  Data-parallel
over N across 8 cores, coords replicated.

PE (2 fp16 passes per 512-col PSUM bank):
  score = xh.ch + xl'.cb  where xh = fp16(x), xl' = fp16(x - xh) with contract
  rows 126/127 replaced by ones, and cb = fp16(c) with rows 126/127 replaced
  by the hi/lo split of -0.5*||c||^2.  (The dropped xh.cl cross term and the
  two dropped xl rows shift scores by ~1e-3; measured against the fp64
  reference this flips 85/65536 argmaxes = 8.5e-4 harness rel err, 23x under
  the 2e-2 gate.)

Argmax (the old DVE bottleneck, max8+find_index8 = 2 full fp32 scans):
  - ACT copies PSUM->SBUF as fp16 (sc16), so DVE tensor ops run in 2x mode
  - DVE: 4-level tensor_max tree over 16-elem groups -> gmax[128,256]
    (2048+512+256+256 cycles instead of a 4096-cycle 1x scan)
  - DVE: max8 + max_index on gmax (256) -> max m + first group g* holding it
  - GPSIMD ap_gather: gather group g* (16 elems) for every row (indices are
    shared per 16-partition cluster, so each row receives all 16 cluster
    rows' groups = 256 gathered values, all slices of its own score row; its
    own argmax group is always among them)
  - DVE: max_index over the gathered 256 -> position of m; host decodes k.
All value comparisons happen on the same fp16-rounded scores (monotone), so
the pipeline is self-consistent; ties resolve to the first occurrence except
across exact fp16 ties (counted in the 85 flips above).

Built with bacc.Bacc + compile() so multi-wait instructions are legalized.
NOTE: tensor_tensor_reduce is avoided - it wedges TRN2 (NRT unrecoverable).
NOTE: fp32r matmul is avoided - TF32-class precision flips argmins.
NOTE: InstPool is avoided - walrus codegen rejects its 5D AP encoding
      (is_valid_s4d4_pl_addr) on this toolchain.
NOTE: gpsimd indirect_copy is avoided - it dies on HW (INTERNAL error on
      result fetch); ap_gather is the working path.
NOTE: gpsimd tensor_tensor is avoided - TENSOR_TENSOR is not a legal Pool
      engine opcode (walrus NCC_IXCG966); GPSIMD also cannot access PSUM.

Measured on trn2 (8 cores): 297,476 ns HW exec (baseline max8+find_index8
fp32 design: 602,244 ns -> 2.02x).  130/65536 index mismatches = 1.88e-3
harness rel err (gate 2e-2).  Engine busy per 4.65us tile slot: DVE 4.05us
(pacer, zero steady-state gaps), ACT 3.87us (2 PSUM->SBUF fp16 copies -
the structural floor), PE 3.8us (16 matmuls), GPSIMD ~1us (gather).
Pipeline head ~24us, tail ~9us.
"""

import numpy as np

N, K, D = 65536, 4096, 128
NCORES = 8
NSHARD = N // NCORES          # 8192 rows per core
NTILES = NSHARD // 128        # 64 tiles of 128 rows
KHALF = K // 2                # 2048 = 4 PSUM banks
GSIZE = 16                    # argmax group size
NGRP = K // GSIZE             # 256 groups


def build_program(ntiles=NTILES):
    import concourse.mybir as mybir
    import concourse.tile as tile
    from concourse import bacc

    f32 = mybir.dt.float32
    f16 = mybir.dt.float16
    i16 = mybir.dt.int16
    u16 = mybir.dt.uint16

    nshard = ntiles * 128
    nc = bacc.Bacc("TRN2", target_bir_lowering=False, debug=False)
    xh_d = nc.dram_tensor("xh", [D, nshard], f16, kind="ExternalInput").ap()
    xo_d = nc.dram_tensor("xo", [D, nshard], f16, kind="ExternalInput").ap()
    ch_d = nc.dram_tensor("ch", [D, K], f16, kind="ExternalInput").ap()
    cb_d = nc.dram_tensor("cb", [D, K], f16, kind="ExternalInput").ap()
    outg = nc.dram_tensor("gidx", [128, ntiles * 8], u16, kind="ExternalOutput").ap()
    outp = nc.dram_tensor("pos", [128, ntiles * 8], u16, kind="ExternalOutput").ap()

    def grp(ap, j):
        return ap.rearrange("p (g j) -> p g j", j=j)

    with tile.TileContext(nc) as tc:
        with (
            tc.tile_pool(name="const", bufs=1) as constp,
            tc.tile_pool(name="xin", bufs=4) as xinp,
            tc.tile_pool(name="psum", bufs=2, space="PSUM") as psump,
            tc.tile_pool(name="scores", bufs=3) as scp,
            tc.tile_pool(name="tree", bufs=3) as treep,
            tc.tile_pool(name="gmax", bufs=3) as gmaxp,
            tc.tile_pool(name="m8", bufs=3) as m8p,
            tc.tile_pool(name="gat", bufs=3) as gatp,
            tc.tile_pool(name="iall", bufs=1) as iallp,
        ):
            # tile 0's inputs first in the DMA queue, then the first column
            # chunk of each constant, then the rest: the first matmuls need
            # only xh0/xo0 + chunk 0, so PE starts ~1.5us in instead of
            # waiting for the full 2MB constant load.
            xh0 = xinp.tile([D, 128], f16, tag="xh")
            nc.sync.dma_start(xh0[:], xh_d[:, 0:128])
            xo0 = xinp.tile([D, 128], f16, tag="xo")
            nc.sync.dma_start(xo0[:], xo_d[:, 0:128])
            ch_sb = constp.tile([D, K], f16)
            cb_sb = constp.tile([D, K], f16)
            nc.sync.dma_start(ch_sb[:, 0:512], ch_d[:, 0:512])
            nc.sync.dma_start(cb_sb[:, 0:512], cb_d[:, 0:512])
            for b in range(1, 8):
                s = slice(b * 512, (b + 1) * 512)
                nc.sync.dma_start(ch_sb[:, s], ch_d[:, s])
                nc.sync.dma_start(cb_sb[:, s], cb_d[:, s])
            gall = iallp.tile([128, ntiles * 8], u16)
            pall = iallp.tile([128, ntiles * 8], u16)

            for i in range(ntiles):
                if i == 0:
                    xh, xo = xh0, xo0
                else:
                    xh = xinp.tile([D, 128], f16, tag="xh")
                    nc.sync.dma_start(xh[:], xh_d[:, i * 128:(i + 1) * 128])
                    xo = xinp.tile([D, 128], f16, tag="xo")
                    nc.sync.dma_start(xo[:], xo_d[:, i * 128:(i + 1) * 128])
                sc = scp.tile([128, K], f16)
                for half in range(2):
                    ps = psump.tile([128, KHALF], f32)
                    koff = half * KHALF
                    for b in range(4):
                        pb = ps[:, b * 512:(b + 1) * 512]
                        co = koff + b * 512
                        nc.tensor.matmul(pb, xh[:], ch_sb[:, co:co + 512],
                                         start=True, stop=False)
                        nc.tensor.matmul(pb, xo[:], cb_sb[:, co:co + 512],
                                         start=False, stop=True)
                    if i == 0:
                        # fill the pipe faster on the first tile: quarter
                        # copies start after 2 matmuls instead of 8
                        for q in range(2):
                            qo = q * (KHALF // 2)
                            nc.scalar.copy(
                                sc[:, koff + qo:koff + qo + KHALF // 2],
                                ps[:, qo:qo + KHALF // 2],
                            )
                    else:
                        nc.scalar.copy(sc[:, koff:koff + KHALF], ps[:])
                t1 = treep.tile([128, K // 2], f16, tag="t1")
                if i == 0:
                    for h in range(2):
                        hg = slice(h * (NGRP // 2), (h + 1) * (NGRP // 2))
                        nc.vector.tensor_max(
                            grp(t1[:], 8)[:, hg, :],
                            grp(sc[:], 16)[:, hg, 0:8],
                            grp(sc[:], 16)[:, hg, 8:16],
                        )
                else:
                    nc.vector.tensor_max(
                        grp(t1[:], 8), grp(sc[:], 16)[:, :, 0:8],
                        grp(sc[:], 16)[:, :, 8:16]
                    )
                t2 = treep.tile([128, K // 4], f16, tag="t2")
                nc.vector.tensor_max(
                    grp(t2[:], 4), grp(t1[:], 8)[:, :, 0:4], grp(t1[:], 8)[:, :, 4:8]
                )
                t3 = treep.tile([128, K // 8], f16, tag="t3")
                nc.vector.tensor_max(
                    grp(t3[:], 2), grp(t2[:], 4)[:, :, 0:2], grp(t2[:], 4)[:, :, 2:4]
                )
                gmax = gmaxp.tile([128, NGRP], f16)
                nc.vector.tensor_max(
                    grp(gmax[:], 1), grp(t3[:], 2)[:, :, 0:1], grp(t3[:], 2)[:, :, 1:2]
                )
                m8 = m8p.tile([128, 8], f16)
                nc.vector.max(m8[:], gmax[:])
                nc.vector.max_index(gall[:, i * 8:(i + 1) * 8], m8[:], gmax[:])
                gat = gatp.tile([128, NGRP], f16)
                nc.gpsimd.ap_gather(
                    grp(gat[:], GSIZE), grp(sc[:], GSIZE),
                    gall[:, i * 8:i * 8 + 1].bitcast(i16),
                    channels=128, num_elems=NGRP, d=GSIZE, num_idxs=GSIZE,
                )
                nc.vector.max_index(pall[:, i * 8:(i + 1) * 8], m8[:], gat[:])
                if i == ntiles // 2 - 1:
                    # overlap half the output writeback with remaining tiles
                    mid = (ntiles // 2) * 8
                    nc.sync.dma_start(outg[:, 0:mid], gall[:, 0:mid])
                    nc.sync.dma_start(outp[:, 0:mid], pall[:, 0:mid])
            mid = (ntiles // 2) * 8
            nc.sync.dma_start(outg[:, mid:], gall[:, mid:])
            nc.sync.dma_start(outp[:, mid:], pall[:, mid:])
    nc.compile()
    return nc


def make_inputs(latent, coords):
    latent = np.asarray(latent, dtype=np.float32)
    coords = np.asarray(coords, dtype=np.float32)
    xT = np.ascontiguousarray(latent.T)                      # [128, N] f32
    cT = np.ascontiguousarray(coords.T)                      # [128, K] f32
    xh = xT.astype(np.float16)
    xl = (xT - xh.astype(np.float32)).astype(np.float16)
    ch = cT.astype(np.float16)
    c2 = (coords * coords).sum(axis=1, dtype=np.float32)     # [K]
    bias = (-0.5 * c2).astype(np.float32)
    bh = bias.astype(np.float16)
    bl = (bias - bh.astype(np.float32)).astype(np.float16)
    # xo: xl with contract rows 126/127 replaced by ones (bias carriers)
    xo = xl.copy()
    xo[126:128, :] = np.float16(1.0)
    # cb: ch with rows 126/127 replaced by bias hi/lo
    cb = ch.copy()
    cb[126, :] = bh
    cb[127, :] = bl
    in_maps = []
    for c in range(NCORES):
        s = slice(c * NSHARD, (c + 1) * NSHARD)
        in_maps.append({
            "xh": np.ascontiguousarray(xh[:, s]),
            "xo": np.ascontiguousarray(xo[:, s]),
            "ch": ch, "cb": cb,
        })
    return in_maps


def gather_output(results, ntiles=NTILES):
    outs = []
    for c in range(NCORES):
        g = np.asarray(results[c]["gidx"]).reshape(128, ntiles, 8)[:, :, 0]
        pos = np.asarray(results[c]["pos"]).reshape(128, ntiles, 8)[:, :, 0]
        i = (pos >> 4).astype(np.int64)                      # which row's group
        j = (pos & 15).astype(np.int64)                      # offset within group
        p = np.arange(128)[:, None]
        donor = (p // 16) * 16 + i                           # [128, ntiles]
        grp = g[donor, np.arange(ntiles)[None, :]].astype(np.int64)
        k = grp * GSIZE + j                                  # [128, ntiles]
        outs.append(k.T.reshape(-1))                         # shard-row order
    return np.concatenate(outs).astype(np.int32)


_NC_CACHE = None


def kernel(latent, coords):
    global _NC_CACHE
    from concourse import bass_utils

    if _NC_CACHE is None:
        _NC_CACHE = build_program()
    in_maps = make_inputs(latent, coords)
    res = bass_utils.run_bass_kernel_spmd(
        _NC_CACHE, in_maps, core_ids=list(range(NCORES))
    )
    return gather_output(res.results)
